# revision 46
# baseline (speedup 1.0000x reference)
"""Causal self-attention (RoPE, GQA) on 8 Trainium2 NeuronCores.

Sharding: 2-way data-parallel over batch x 4-way tensor-parallel over heads.
Core c handles batch c//4 and head-group c%4 (4 q-heads, 2 kv-heads).
Each core computes its partial output projection (wo row-shard); the host
sums the 4 partials per batch (the "all-reduce" happens in the unshard step).

All-bf16 datapath (fp8 was measured to break the 2e-2 tolerance):
  - x / wq / wk / wv / wo shipped as bf16 (halves HBM traffic vs fp32),
    output written bf16 and summed fp32 on the host.
  - Layouts all transposed so no on-device transposes are needed:
    Q/K produced as [head_dim, S] via lhsT=weight-slice, rhs=xT; V as
    [S, dv] via lhsT=xT-slice. RoPE: weight rows pre-permuted (even
    components -> rows 0:64, odd -> 64:128) so rotation is elementwise
    on row halves.
  - Phase 2 pipelines per k-block: one score matmul per head into a
    shared [128, 2, QC] PSUM tile (plane i = head i of the pair, same
    k-block), ONE exp over both planes, then PV + Z matmuls two
    iterations later (software pipeline) so the PE never waits on ACT.
    Heads of a pair share one kv head => score/PV LDWEIGHTS amortize.
  - Z (softmax denominator) per head accumulated into partitions 0/32
    of a single PSUM bank via ones-matmul col-tiling; 1/Z broadcast to
    128 partitions with a K=1 matmul; O^T normalized on DVE.
  - Output projection for a q-chunk's 4 s-blocks emitted right after
    its heads finish, reusing the score PSUM buffers (same pool tag).

Scheduling constraint honored throughout: a DVE TensorTensor can carry at
most ONE sync-wait, so every TT here has at most one freshly-produced
cross-engine operand (constants are "warmed" with a dummy DVE touch).
"""

import sys
import numpy as np
import ml_dtypes

sys.path.insert(0, "/opt/trn_rl_repo")

import concourse.bass as bass
import concourse.bacc as bacc
import concourse.mybir as mybir
from concourse import tile
from concourse.bass_utils import run_bass_kernel_spmd

F32 = mybir.dt.float32
F32R = mybir.dt.float32r
BF16 = mybir.dt.bfloat16
AF = mybir.ActivationFunctionType
OP = mybir.AluOpType

B, S, D = 2, 2048, 2048
HQ, HKV, HD = 16, 8, 128
ROPE_THETA = 10000.0
NCORES, TP = 8, 4
HQL, HKL = HQ // TP, HKV // TP        # 4 q heads, 2 kv heads per core
NKT = D // 128                        # 16 contraction tiles
QC = 512                              # q-chunk width
NQC = S // QC                         # 4 q chunks
NSB = S // 128                        # 16 s-blocks
SCALE = 1.0 / float(np.sqrt(HD))
BF = ml_dtypes.bfloat16


def _build_nc():
    nc = bacc.Bacc("TRN2", target_bir_lowering=False)

    xT_d = nc.dram_tensor("xT", [NKT, 128, S], BF16, kind="ExternalInput")
    wq_d = nc.dram_tensor("wq_t", [NKT, 128, HQL * HD], BF16, kind="ExternalInput")
    wk_d = nc.dram_tensor("wk_t", [NKT, 128, HKL * HD], BF16, kind="ExternalInput")
    wv_d = nc.dram_tensor("wv_t", [NKT, 128, HKL * HD], BF16, kind="ExternalInput")
    wo_d = nc.dram_tensor("wo_t", [HQL, 128, D], BF16, kind="ExternalInput")
    cos_d = nc.dram_tensor("cos128", [128, S], BF16, kind="ExternalInput")
    sin_d = nc.dram_tensor("sinM", [128, S], BF16, kind="ExternalInput")
    mask_d = nc.dram_tensor("dmask", [4, 128, 2, QC], BF16, kind="ExternalInput")
    out_d = nc.dram_tensor("out", [NSB, 128, D], BF16, kind="ExternalOutput")

    with tile.TileContext(nc) as tc:
        with (
            tc.tile_pool(name="resident", bufs=1) as res,
            tc.tile_pool(name="xstream", bufs=3) as xpool,
            tc.tile_pool(name="ropetmp", bufs=2) as rtmp,
            tc.tile_pool(name="epool", bufs=4) as epool,
            tc.tile_pool(name="small", bufs=2) as small,
            tc.tile_pool(name="outp", bufs=2) as outp,
        ):
            # ---------- resident tiles (weights split per k-tile so the
            # first matmuls start after ~0.5MB of DMA) ----------
            wq_sb = [res.tile([128, HQL * HD], BF16, tag=f"wq{kt}", name=f"wq{kt}")
                     for kt in range(NKT)]
            wk_sb = [res.tile([128, HKL * HD], BF16, tag=f"wk{kt}", name=f"wk{kt}")
                     for kt in range(NKT)]
            wv_sb = [res.tile([128, HKL * HD], BF16, tag=f"wv{kt}", name=f"wv{kt}")
                     for kt in range(NKT)]
            wo_sb = res.tile([128, HQL, D], BF16)
            cos_sb = res.tile([128, S], BF16)
            sin_sb = res.tile([128, S], BF16)
            mask_sb = res.tile([128, 4, 2, QC], BF16)

            ones_bf = res.tile([128, 1], BF16)
            nc.vector.memset(ones_bf[:], 1.0)
            ones_rf = res.tile([1, 128], F32)
            nc.vector.memset(ones_rf[:], 1.0)
            ones_r = res.tile([1, 128], F32R)
            nc.vector.tensor_copy(ones_r[:], ones_rf[:])

            def pe_warm(n):
                # dummy LDWEIGHTS: no PSUM writes, no deps beyond ones_bf.
                # Keeps the PE "active" through DMA waits / pool barriers so
                # the HAM clock gate stays at 8/8 (a >3.4us idle re-throttles
                # the PE to half clock for the next ~3.4us of work).
                for _ in range(n):
                    nc.tensor.ldweights(ones_bf[:])

            pe_warm(128)

            # outputs of phase 1 (resident through phase 2/3)
            qt_sb = [res.tile([128, S], BF16, tag=f"qt{h}", name=f"qt{h}") for h in range(HQL)]
            kt_sb = [res.tile([128, S], BF16, tag=f"kt{h}", name=f"kt{h}") for h in range(HKL)]
            v_sb = res.tile([128, NSB, HKL * HD], BF16)
            ot_sb = [res.tile([128, S], BF16, tag=f"ot{h}", name=f"ot{h}") for h in range(HQL)]

            # ---------- phase 1: QKV projection + RoPE ----------
            late_ropes = []
            with tc.tile_pool(name="ps1", bufs=1, space="PSUM") as ps1:
                for qc in range(NQC):
                    qsl = slice(qc * QC, (qc + 1) * QC)
                    qps = [ps1.tile([128, QC], F32, tag=f"qps{h}", name=f"qps{h}_{qc}") for h in range(HQL)]
                    kps = [ps1.tile([128, QC], F32, tag=f"kps{h}", name=f"kps{h}_{qc}") for h in range(HKL)]
                    vps = ps1.tile([128, 4, HKL * HD], F32, tag="vps")
                    for kt in range(NKT):
                        if qc == 0:
                            # stream weights alongside the first x chunk on the
                            # second HWDGE queue (scalar) so they don't delay x
                            nc.scalar.dma_start(wq_sb[kt][:], wq_d[kt, :, :])
                            nc.scalar.dma_start(wk_sb[kt][:], wk_d[kt, :, :])
                            nc.scalar.dma_start(wv_sb[kt][:], wv_d[kt, :, :])
                        xt = xpool.tile([128, QC], BF16)
                        nc.sync.dma_start(xt[:], xT_d[kt, :, qsl])
                        if qc == 0 and kt == 8:
                            # tables needed from the first drain onward
                            nc.scalar.dma_start(cos_sb[:], cos_d[:])
                            nc.scalar.dma_start(sin_sb[:], sin_d[:])
                        if qc == 3 and kt == 0:
                            # wo/mask are phase-2-only: stream them last so
                            # they never compete with the x/weight streams
                            nc.scalar.dma_start(mask_sb[:],
                                                mask_d.rearrange("m p j c -> p m j c"))
                            nc.scalar.dma_start(wo_sb[:],
                                                wo_d.rearrange("h p m -> p h m"))
                        st, sp = (kt == 0), (kt == NKT - 1)
                        for h in range(HQL):
                            nc.tensor.matmul(qps[h][:], wq_sb[kt][:, h * HD:(h + 1) * HD],
                                             xt[:], start=st, stop=sp)
                        for h in range(HKL):
                            nc.tensor.matmul(kps[h][:], wk_sb[kt][:, h * HD:(h + 1) * HD],
                                             xt[:], start=st, stop=sp)
                        for sb in range(4):
                            # two 256-col outputs share one PSUM bank: only the
                            # bank's first writer may clear has_written (start)
                            nc.tensor.matmul(vps[:, sb, :], xt[:, sb * 128:(sb + 1) * 128],
                                             wv_sb[kt][:],
                                             start=(st and sb % 2 == 0), stop=sp,
                                             skip_group_check=True)
                        if qc == 0 and kt < 12:
                            # weight stream races compute here; keep the PE
                            # warm across the per-kt DMA waits
                            pe_warm(8)

                    # drain in next-qc consumption order (v, k, q): v copies
                    # first on DVE, k evacs on ACT, q evacs on DVE; RoPE runs
                    # SBUF-side in bf16 on the DVE fast modes. For the LAST qc
                    # only k0/q0/q1 rope inline (phase 2's first scores read
                    # them); k1/q2/q3 ropes are deferred into phase 2 so the
                    # phase-transition barrier doesn't wait for them.
                    last = (qc == NQC - 1)
                    units = ([(qps[i], qt_sb[i]) for i in range(HQL)]
                             + [(kps[i], kt_sb[i]) for i in range(HKL)])
                    if last:
                        units = units[HQL:] + units[:HQL]
                    evacs = []
                    for i, (ps, dst) in enumerate(units):
                        qsb = rtmp.tile([128, QC], BF16, tag="evac",
                                        name=f"evac{qc}_{i}")
                        if i in (2, 3):
                            nc.vector.tensor_copy(qsb[:], ps[:])
                        else:
                            nc.scalar.copy(qsb[:], ps[:])
                        evacs.append((qsb, dst))
                    for sb in range(4):
                        nc.vector.tensor_copy(v_sb[:, qc * 4 + sb, :],
                                              vps[:, sb, :])

                    def rope(qsb, dst, i, qsl=qsl, qc=qc):
                        qsw = rtmp.tile([128, QC], BF16, tag="swap",
                                        name=f"swap{qc}_{i}")
                        nc.vector.tensor_copy(qsw[0:64, :], qsb[64:128, :])
                        nc.vector.tensor_copy(qsw[64:128, :], qsb[0:64, :])
                        a_t = rtmp.tile([128, QC], BF16, tag="ropeA")
                        nc.vector.tensor_tensor(a_t[:], qsb[:], cos_sb[:, qsl], OP.mult)
                        b_t = rtmp.tile([128, QC], BF16, tag="ropeB")
                        nc.vector.tensor_tensor(b_t[:], qsw[:], sin_sb[:, qsl], OP.mult)
                        nc.vector.tensor_tensor(dst[:, qsl], a_t[:], b_t[:], OP.add)

                    for i, (qsb, dst) in enumerate(evacs):
                        rope(qsb, dst, i)

            # warm the mask tile on DVE so phase-2 mask TTs carry only the
            # ACT-produced operand's wait
            mwarm = small.tile([1, 8], BF16, tag="mwarm")
            nc.vector.tensor_copy(mwarm[:], mask_sb[0:1, 0, 0, 0:8])

            # the phase-1 -> phase-2 PSUM pool swap barriers on the old
            # pool's consumer chain (~5-9us of DVE rope tail); these dummy
            # loads execute during that wait and keep the HAM gate open
            pe_warm(160)

            # ---------- phase 2: attention + interleaved output proj ----------
            # Finalize work (1/Z chain + normalize, and each qc's output
            # projection) is DEFERRED into the next head-pair's stream, two
            # score-iterations in, so the PE never waits on the DVE chain.
            with (
                tc.tile_pool(name="ps_sps", bufs=2, space="PSUM") as ps_sps,
                tc.tile_pool(name="ps_ops", bufs=1, space="PSUM") as ps_ops,
                tc.tile_pool(name="ps_z", bufs=1, space="PSUM") as ps_z,
            ):
                def make_finalizer(qc, hp, ops, zps, heads, qsl, use_rb_mm):
                    def fin():
                        # 1/Z per head -> broadcast -> normalize O^T
                        for i, h in enumerate(heads):
                            rz = small.tile([1, QC], F32, tag="rz",
                                            name=f"rz{qc}_{hp}_{i}")
                            nc.vector.reciprocal_approx_fast(
                                rz[:], zps[0:1, i, :])
                            rb_sb = small.tile([128, QC], F32, tag="rbsb",
                                               name=f"rbsb{qc}_{hp}_{i}")
                            if use_rb_mm:
                                # tail path: broadcast via K=1 matmul on the
                                # (otherwise idle) PE to shorten the chain
                                rz_r = small.tile([1, QC], F32R, tag="rzr",
                                                  name=f"rzr{qc}_{hp}_{i}")
                                nc.vector.tensor_copy(rz_r[:], rz[:])
                                rbp = ps_sps.tile([128, 2, QC], F32, tag="sps",
                                                  name=f"rbp{qc}_{hp}_{i}")
                                nc.tensor.matmul(rbp[:, 0, :], ones_r[:],
                                                 rz_r[:], start=True, stop=True)
                                nc.vector.tensor_copy(rb_sb[:], rbp[:, 0, :])
                            else:
                                nc.gpsimd.partition_broadcast(rb_sb[:], rz[:])
                                # warm rb_sb on DVE so the normalize TT carries
                                # only the PSUM operand's wait
                                rwarm = small.tile([1, 8], F32, tag="rwarm",
                                                   name=f"rwarm{qc}_{hp}_{i}")
                                nc.vector.tensor_copy(rwarm[:], rb_sb[0:1, 0:8])
                            nc.vector.tensor_tensor(ot_sb[h][:, qsl],
                                                    ops[:, i, :],
                                                    rb_sb[:], OP.mult)
                    return fin

                def make_proj_units(qc):
                    # one thunk per (s-block, d-half) so the units can be
                    # spread across the next head-pair's iterations
                    units = []
                    for sb in range(4 * qc, 4 * qc + 4):
                        for dcp in range(2):
                            def unit(sb=sb, dcp=dcp):
                                fps = ps_sps.tile([128, 2, QC], F32, tag="sps",
                                                  name=f"fps{sb}_{dcp}")
                                for h in range(HQL):
                                    otb = ot_sb[h][:, sb * 128:(sb + 1) * 128]
                                    for j in range(2):
                                        dc = 2 * dcp + j
                                        nc.tensor.matmul(
                                            fps[:, j, :], otb,
                                            wo_sb[:, h, dc * QC:(dc + 1) * QC],
                                            start=(h == 0), stop=(h == HQL - 1))
                                o_sb = outp.tile([128, 2, QC], BF16, tag="osb")
                                nc.vector.tensor_copy(
                                    o_sb[:].rearrange("p a b -> p (a b)"),
                                    fps[:].rearrange("p a b -> p (a b)"))
                                # always the sync ring: scalar-issued DMAs
                                # would occupy the ACT queue and stall exps
                                nc.sync.dma_start(
                                    out_d[sb, :, dcp * 2 * QC:(dcp + 1) * 2 * QC],
                                    o_sb[:].rearrange("p a b -> p (a b)"))
                            units.append(unit)
                    return units

                deferred = list(late_ropes)
                for qc in range(NQC):
                    qsl = slice(qc * QC, (qc + 1) * QC)
                    for hp in range(2):
                        h0, h1 = 2 * hp, 2 * hp + 1
                        kv = hp
                        nkb = 4 * qc + 4
                        ops = ps_ops.tile([128, 2, QC], F32, tag="ops",
                                          name=f"ops{qc}_{hp}")
                        # one PSUM bank per head (both at partition 0; the
                        # col-tiled partition-32 variant miswrites on HW)
                        zps = ps_z.tile([1, 2, QC], F32, tag="zps",
                                        name=f"zps{qc}_{hp}")

                        pend = []

                        def flush_one(ops=ops, zps=zps, kv=kv, nkb=nkb, pend=pend):
                            # (a matmul output may not cross a PSUM bank
                            # boundary, so each head/plane gets its own MM)
                            kb, off, e_t = pend.pop(0)
                            sp_ = (kb == nkb - 1)
                            vblk = v_sb[:, kb, kv * HD:(kv + 1) * HD]
                            for i in range(2):
                                nc.tensor.matmul(
                                    ops[:, i, off:], vblk, e_t[:, i, off:],
                                    start=(kb == 0), stop=sp_,
                                    skip_group_check=True)
                                nc.tensor.matmul(
                                    zps[0:1, i, off:], ones_bf[:],
                                    e_t[:, i, off:], start=(kb == 0), stop=sp_,
                                    skip_group_check=True)

                        for kb in range(nkb):
                            off = (kb - 4 * qc) * 128 if kb >= 4 * qc else 0
                            sps = ps_sps.tile([128, 2, QC], F32, tag="sps",
                                              name=f"sps{qc}_{hp}_{kb}")
                            for i, h in enumerate((h0, h1)):
                                nc.tensor.matmul(
                                    sps[:, i, off:],
                                    kt_sb[kv][:, kb * 128:(kb + 1) * 128],
                                    qt_sb[h][:, qc * QC + off:(qc + 1) * QC],
                                    start=True, stop=True)
                            e_t = epool.tile([128, 2, QC], BF16, tag="et",
                                             name=f"et{qc}_{hp}_{kb}")
                            nc.scalar.activation(e_t[:, :, off:], sps[:, :, off:],
                                                 AF.Exp, scale=SCALE)
                            if kb >= 4 * qc:
                                m = kb - 4 * qc
                                nc.vector.tensor_tensor(
                                    e_t[:, :, off:], e_t[:, :, off:],
                                    mask_sb[:, m, :, off:], OP.mult)
                            pend.append((kb, off, e_t))
                            if kb >= 1 and deferred:
                                deferred.pop(0)()
                            if len(pend) > 2:
                                flush_one()
                        while deferred:
                            deferred.pop(0)()
                        while pend:
                            flush_one()

                        deferred.append(make_finalizer(qc, hp, ops, zps,
                                                       (h0, h1), qsl,
                                                       use_rb_mm=(qc == NQC - 1
                                                                  and hp == 1)))
                    deferred.extend(make_proj_units(qc))
                for f in deferred:
                    f()

    nc.compile()
    return nc


_NC_CACHE = None


def _get_nc():
    global _NC_CACHE
    if _NC_CACHE is None:
        _NC_CACHE = _build_nc()
    return _NC_CACHE


def _rope_tables():
    inv = 1.0 / (ROPE_THETA ** (np.arange(0, HD, 2, dtype=np.float64) / HD))  # [64]
    t = np.arange(S, dtype=np.float64)
    ang = np.outer(inv, t)                      # [64, S]
    cos = np.cos(ang).astype(np.float32)
    sin = np.sin(ang).astype(np.float32)
    cos128 = np.concatenate([cos, cos], axis=0).astype(BF)  # [128, S]
    sinM = np.concatenate([-sin, sin], axis=0).astype(BF)
    return cos128, sinM


def _masks():
    # dmask[m][p, j, c] = (m*128 + p <= c); both j planes identical (the two
    # planes of an e-tile hold two heads at the SAME k-block)
    p = np.arange(128)
    c = np.arange(QC)
    m = np.zeros((4, 128, 2, QC), np.float32)
    for mi in range(4):
        valid = ((mi * 128 + p)[:, None] <= c[None, :]).astype(np.float32)
        m[mi, :, 0, :] = valid
        m[mi, :, 1, :] = valid
    return m.astype(BF)


def prepare_inputs(x, wq, wk, wv, wo):
    """Build the 8 per-core input dicts from full inputs."""
    perm = np.concatenate([np.arange(0, HD, 2), np.arange(1, HD, 2)])
    cos128, sinM = _rope_tables()
    dmask = _masks()

    x = np.asarray(x, np.float32)
    wq = np.asarray(wq, np.float32).reshape(HQ, HD, D)[:, perm, :]
    wk = np.asarray(wk, np.float32).reshape(HKV, HD, D)[:, perm, :]
    wv = np.asarray(wv, np.float32).reshape(HKV, HD, D)
    wo = np.asarray(wo, np.float32)              # [D, HQ*HD]

    in_maps = []
    for c in range(NCORES):
        b, hg = divmod(c, TP)
        qh = slice(hg * HQL, (hg + 1) * HQL)
        kh = slice(hg * HKL, (hg + 1) * HKL)
        xT = np.ascontiguousarray(x[b].T).reshape(NKT, 128, S).astype(BF)
        wq_t = np.ascontiguousarray(
            wq[qh].reshape(HQL * HD, D).T).reshape(NKT, 128, HQL * HD).astype(BF)
        wk_t = np.ascontiguousarray(
            wk[kh].reshape(HKL * HD, D).T).reshape(NKT, 128, HKL * HD).astype(BF)
        wv_t = np.ascontiguousarray(
            wv[kh].reshape(HKL * HD, D).T).reshape(NKT, 128, HKL * HD).astype(BF)
        wo_t = np.ascontiguousarray(
            wo[:, hg * HQL * HD:(hg + 1) * HQL * HD].T.reshape(HQL, HD, D)
        ).astype(BF)
        in_maps.append({
            "xT": xT, "wq_t": wq_t, "wk_t": wk_t, "wv_t": wv_t, "wo_t": wo_t,
            "cos128": cos128, "sinM": sinM, "dmask": dmask,
        })
    return in_maps


def _install_ntff_hook():
    """The agent image's antenv lacks axon_hooks; synthesize it so
    run_bass_kernel_spmd(trace=True) can capture NTFF profiles."""
    import sys as _sys
    import types, contextlib, ctypes

    if "antenv.axon_hooks" in _sys.modules:
        return
    so_path = "/opt/axon/libaxon_pjrt.so"
    lib = ctypes.CDLL(so_path)
    if not hasattr(lib, "axon_start_nrt_profile"):
        return
    lib.axon_start_nrt_profile.argtypes = [ctypes.POINTER(ctypes.c_int64),
                                           ctypes.c_size_t]
    lib.axon_start_nrt_profile.restype = ctypes.c_int64
    lib.axon_stop_nrt_profile.argtypes = [ctypes.c_char_p]
    lib.axon_stop_nrt_profile.restype = ctypes.c_int64

    @contextlib.contextmanager
    def _hook(output_dir, device_ids):
        import jax
        jax.devices()
        if device_ids:
            ids = (ctypes.c_int64 * len(device_ids))(*device_ids)
            rc = lib.axon_start_nrt_profile(ids, len(device_ids))
        else:
            rc = lib.axon_start_nrt_profile(None, 0)
        if rc != 0:
            raise RuntimeError(f"axon_start_nrt_profile rc={rc}")
        try:
            yield
        finally:
            n = lib.axon_stop_nrt_profile(str(output_dir).encode())
            print(f"ntff profile: {n} file(s) written to {output_dir}",
                  file=_sys.stderr)

    mod = types.ModuleType("antenv.axon_hooks")
    mod.get_axon_ntff_profile_hook = lambda: _hook
    mod.set_axon_ntff_profile_hook = lambda h: None
    _sys.modules["antenv.axon_hooks"] = mod
    try:
        import antenv
        antenv.axon_hooks = mod
    except ImportError:
        pass


def kernel(x, wq, wk, wv, wo, _trace=False, _trace_cores=None):
    in_maps = prepare_inputs(x, wq, wk, wv, wo)
    if _trace:
        _install_ntff_hook()
    nc = _get_nc()
    res = run_bass_kernel_spmd(
        nc, in_maps, core_ids=list(range(NCORES)),
        trace=_trace, trace_cores=_trace_cores)
    out = np.zeros((B, S, D), np.float32)
    for c in range(NCORES):
        b = c // TP
        out[b] += res.results[c]["out"].reshape(S, D).astype(np.float32)
    kernel.last_results = res
    return out


if __name__ == "__main__":
    rng = np.random.default_rng(0)
    x = rng.standard_normal((B, S, D), dtype=np.float32)
    sc = 1.0 / np.sqrt(D)
    wq = (rng.standard_normal((HQ * HD, D), dtype=np.float32) * sc)
    wk = (rng.standard_normal((HKV * HD, D), dtype=np.float32) * sc)
    wv = (rng.standard_normal((HKV * HD, D), dtype=np.float32) * sc)
    wo = (rng.standard_normal((D, HQ * HD), dtype=np.float32) * sc)
    out = kernel(x, wq, wk, wv, wo)
    print("ran", out.shape, out.dtype, float(np.abs(out).mean()))
